# revision 10
# baseline (speedup 1.0000x reference)
# KNN-impute column kernel for Trainium2 (Bass/Tile), 8-core data parallel.
#
# Problem (single imputed column, COL=0):
#   For each of Nq=4096 query rows: find the K=5 smallest distances among
#   the "potential" donor columns of dist_chunk[q, :Nt] (Nt=16384), weight
#   donors by 1/dist, output weighted mean into column 0 of X for rows
#   where the value is missing (receiver mask).
#
# Device strategy per core (512 rows = 4 blocks of 128 partitions):
#   - gpsimd:  dneg = pen_rep - d   (pen = 0 for valid donor col, -inf for
#              invalid) computed in place over the [128, 16384] block tile.
#   - DVE:     max(dneg) -> 8 largest = 8 smallest distances (negated),
#              max_index -> their column indices.  Tie semantics match
#              jax.lax.top_k exactly (descending value, ties -> ascending
#              index, duplicates get successive distinct positions).
#   - indirect DMA gathers donor values _fit_X[idx, 0] from HBM.
#   - small-tile epilogue: w = 1/vals (sign cancels in the ratio),
#     knn = sum(w*v)/sum(w), merge into X column 0 under receiver mask.
#
# Host only does O(Nq + Nt) prep (masks, penalty vector, sharding) plus
# degenerate-case fallbacks that cannot occur for the reference data.

import os
import sys

import numpy as np

sys.path.insert(0, "/opt/trn_rl_repo")

COL = 0
K = 5
NQ = 4096
NT = 16384
D = 32
N_CORES = 8
P = 128

_prog_cache = {}


def _build_program(nq_core: int, nt: int):
    """Build the per-core Bass program. All 8 cores run the same program."""
    import concourse.bass as bass
    import concourse.mybir as mybir
    from concourse import bacc, tile

    dt = mybir.dt
    nb = nq_core // P
    assert nq_core % P == 0

    nc = bacc.Bacc(
        "TRN2",
        target_bir_lowering=False,
        debug=False,
        num_devices=N_CORES,
    )

    dist = nc.dram_tensor("dist", [nq_core, nt], dt.float32, kind="ExternalInput")
    xin = nc.dram_tensor("xin", [nq_core, D], dt.float32, kind="ExternalInput")
    recv = nc.dram_tensor("recv", [nq_core], dt.float32, kind="ExternalInput")
    pen = nc.dram_tensor("pen", [1, nt], dt.bfloat16, kind="ExternalInput")
    fitcol = nc.dram_tensor("fitcol", [nt, 1], dt.float32, kind="ExternalInput")
    out = nc.dram_tensor("out", [nq_core, D], dt.float32, kind="ExternalOutput")

    N_DMA_SPLIT = 16
    ch = nt // N_DMA_SPLIT

    with tile.TileContext(nc) as tc:
        with (
            tc.tile_pool(name="bigp", bufs=2) as bigp,
            tc.tile_pool(name="persist", bufs=1) as pp,
            tc.tile_pool(name="small", bufs=1) as sp,
        ):
            # --- penalty vector broadcast to all 128 partitions (bf16) ---
            # broadcast-DMA from DRAM (src partition stride 0); 4 MiB once
            pen_rep = pp.tile([P, nt], dt.bfloat16)
            pen_b = pen.ap().to_broadcast([P, nt])
            for c in range(N_DMA_SPLIT):
                sl = slice(c * ch, (c + 1) * ch)
                nc.sync.dma_start(pen_rep[:, sl], pen_b[:, sl])

            vals_all = sp.tile([P, nb, 8], dt.float32)
            idx_all = sp.tile([P, nb, 8], dt.uint32)

            dist_v = dist.ap().rearrange("(b p) n -> b p n", p=P)

            for b in range(nb):
                dtile = bigp.tile([P, nt], dt.float32, tag="d")
                for c in range(N_DMA_SPLIT):
                    nc.sync.dma_start(
                        dtile[:, c * ch : (c + 1) * ch],
                        dist_v[b, :, c * ch : (c + 1) * ch],
                    )
                # in-place: d <- pen - d   (invalid donors -> -inf)
                # chunked so each instruction waits on few DMA queue sems
                # (walrus limits sync-wait slots per instruction)
                N_TT_CHUNK = 8
                tch = nt // N_TT_CHUNK
                for c in range(N_TT_CHUNK):
                    sl = slice(c * tch, (c + 1) * tch)
                    nc.gpsimd.tensor_tensor(
                        out=dtile[:, sl],
                        in0=pen_rep[:, sl],
                        in1=dtile[:, sl],
                        op=mybir.AluOpType.subtract,
                    )
                nc.vector.max(out=vals_all[:, b, :], in_=dtile[:])
                nc.vector.max_index(
                    out=idx_all[:, b, :],
                    in_max=vals_all[:, b, :],
                    in_values=dtile[:],
                )

            # --- gather donor values for the top-K of every block ---
            # HW indirect DMA consumes ONE offset per partition (gathers a
            # contiguous run per partition), so issue one tiny gather per
            # (block, k) with a single-column offset AP.
            v_all = sp.tile([P, nb, K], dt.float32)
            for b in range(nb):
                for k in range(K):
                    nc.gpsimd.indirect_dma_start(
                        out=v_all[:, b, k : k + 1],
                        out_offset=None,
                        in_=fitcol.ap(),
                        in_offset=bass.IndirectOffsetOnAxis(
                            ap=idx_all[:, b, k : k + 1], axis=0
                        ),
                    )

            # --- epilogue on [P, nb*K] tiles ---
            # w~ = 1/vals = -(1/d); the sign cancels in num/den.
            w_all = sp.tile([P, nb, K], dt.float32)
            nc.vector.reciprocal(w_all[:], vals_all[:, :, :K])
            wv_all = sp.tile([P, nb, K], dt.float32)
            nc.vector.tensor_tensor(
                out=wv_all[:], in0=w_all[:], in1=v_all[:], op=mybir.AluOpType.mult
            )
            den = sp.tile([P, nb], dt.float32)
            num = sp.tile([P, nb], dt.float32)
            nc.vector.tensor_reduce(
                out=den[:], in_=w_all[:], axis=mybir.AxisListType.X,
                op=mybir.AluOpType.add,
            )
            nc.vector.tensor_reduce(
                out=num[:], in_=wv_all[:], axis=mybir.AxisListType.X,
                op=mybir.AluOpType.add,
            )
            # guard den == 0 (all-inf distances row): den <- den + (den == 0)
            eq0 = sp.tile([P, nb], dt.float32)
            nc.vector.tensor_scalar(
                out=eq0[:], in0=den[:], scalar1=0.0, scalar2=None,
                op0=mybir.AluOpType.is_equal,
            )
            nc.vector.tensor_tensor(
                out=den[:], in0=den[:], in1=eq0[:], op=mybir.AluOpType.add
            )
            rden = sp.tile([P, nb], dt.float32)
            nc.vector.reciprocal(rden[:], den[:])
            knn = sp.tile([P, nb], dt.float32)
            nc.vector.tensor_tensor(
                out=knn[:], in0=num[:], in1=rden[:], op=mybir.AluOpType.mult
            )

            # --- merge into X column COL under receiver mask ---
            xt = sp.tile([P, nb, D], dt.float32)
            nc.sync.dma_start(xt[:], xin.ap().rearrange("(b p) c -> p b c", p=P))
            rt = sp.tile([P, nb], dt.float32)
            nc.sync.dma_start(rt[:], recv.ap().rearrange("(b p) -> p b", p=P))

            x0 = xt[:, :, COL]  # strided [P, nb] view of column COL
            # knn <- r * (knn - x0);  x0 <- x0 + that
            nc.vector.tensor_tensor(
                out=knn[:], in0=knn[:], in1=x0, op=mybir.AluOpType.subtract
            )
            nc.vector.tensor_tensor(
                out=knn[:], in0=knn[:], in1=rt[:], op=mybir.AluOpType.mult
            )
            nc.vector.tensor_tensor(
                out=x0, in0=x0, in1=knn[:], op=mybir.AluOpType.add
            )

            nc.sync.dma_start(out.ap().rearrange("(b p) c -> p b c", p=P), xt[:])

    nc.compile()
    return nc


def _get_program(nq_core: int, nt: int):
    key = (nq_core, nt)
    if key not in _prog_cache:
        _prog_cache[key] = _build_program(nq_core, nt)
    return _prog_cache[key]


def _numpy_reference(X, dist_chunk, non_missing_fix_X, mask_fit_X,
                     dist_idx_map, mask, row_missing_idx, _fit_X):
    """Exact numpy port of the jax reference (fallback for degenerate data)."""
    BIG = 1e10
    Nq = X.shape[0]
    col = COL
    potential = non_missing_fix_X[:, col].astype(bool)
    in_missing = np.zeros((Nq,), bool)
    in_missing[row_missing_idx] = True
    receiver = in_missing & mask[:, col].astype(bool)

    d = dist_chunk[dist_idx_map]
    d_pot = np.where(potential[None, :], d, np.inf)
    has_valid = np.any(potential[None, :] & ~np.isnan(d), axis=1)
    all_nan = ~has_valid

    dn = np.where(np.isnan(d_pot), BIG, d_pot)
    # top-k smallest of dn == top-k largest of -dn, stable ties by index
    order = np.argsort(dn, axis=1, kind="stable")
    donors_idx = order[:, :K]
    donors_dist = np.take_along_axis(d_pot, donors_idx, axis=1)

    with np.errstate(divide="ignore", invalid="ignore"):
        w = 1.0 / donors_dist
    inf_mask = np.isinf(w)
    inf_row = np.any(inf_mask, axis=1)
    w = np.where(inf_row[:, None], inf_mask.astype(w.dtype), w)
    w = np.where(np.isnan(w), 0.0, w)

    donors = _fit_X[donors_idx, col]
    donors_mask = 1.0 - mask_fit_X[donors_idx, col].astype(w.dtype)
    valid = potential[donors_idx].astype(w.dtype)
    new_w = donors_mask * w * valid
    ws = np.sum(new_w, axis=1)
    div = np.where(ws == 0, 1.0, ws)
    knn_val = np.sum(donors * new_w, axis=1) / div

    obs = (~mask_fit_X[:, col].astype(bool)).astype(X.dtype)
    msum = np.sum(obs)
    csum = np.sum(obs * _fit_X[:, col])
    col_mean = csum / (msum if msum > 0 else 1.0)

    new_col = np.where(receiver, np.where(all_nan, col_mean, knn_val), X[:, col])
    outX = np.array(X, copy=True)
    outX[:, col] = new_col
    return outX


def _host_prep(X, dist_chunk, non_missing_fix_X, mask_fit_X,
               dist_idx_map, mask, row_missing_idx, _fit_X):
    """Cheap host-side prep. Returns None if data needs the numpy fallback."""
    import ml_dtypes

    Nq = X.shape[0]
    # one fused scan: rejects NaN (NaN > 0 is False) and non-positive
    # distances (reference's inf-weight / NaN paths) in a single pass
    if not (np.asarray(dist_chunk) > 0).all():
        return None
    potential = np.asarray(non_missing_fix_X[:, COL]).astype(bool)
    if not potential.any():
        return None  # all-NaN fallback (column mean) -- cannot happen here

    # d = dist_chunk[dist_idx_map]; identity for the reference data
    idx_map = np.asarray(dist_idx_map)
    if np.array_equal(idx_map, np.arange(Nq, dtype=idx_map.dtype)):
        dist_rows = np.asarray(dist_chunk, dtype=np.float32)
    else:
        dist_rows = np.asarray(dist_chunk, dtype=np.float32)[idx_map]

    in_missing = np.zeros((Nq,), bool)
    in_missing[np.asarray(row_missing_idx)] = True
    receiver = (in_missing & np.asarray(mask[:, COL]).astype(bool)).astype(np.float32)

    pen_f32 = np.where(potential, np.float32(0.0), np.float32(-np.inf))
    pen_bf16 = pen_f32.astype(ml_dtypes.bfloat16).reshape(1, -1)
    fitcol = np.ascontiguousarray(np.asarray(_fit_X[:, COL], dtype=np.float32))
    return dist_rows, receiver, pen_bf16, fitcol


def _run_on_device(shards, trace=False):
    from concourse import bass_utils

    nq_core = NQ // N_CORES
    nc = _get_program(nq_core, NT)
    dist_rows, X, receiver, pen_bf16, fitcol = shards

    in_maps = []
    for c in range(N_CORES):
        sl = slice(c * nq_core, (c + 1) * nq_core)
        in_maps.append({
            "dist": np.ascontiguousarray(dist_rows[sl]),
            "xin": np.ascontiguousarray(np.asarray(X, dtype=np.float32)[sl]),
            "recv": np.ascontiguousarray(receiver[sl]),
            "pen": pen_bf16,
            "fitcol": fitcol.reshape(-1, 1),
        })

    res = bass_utils.run_bass_kernel_spmd(
        nc, in_maps, core_ids=list(range(N_CORES)), trace=trace
    )
    out = np.concatenate([res.results[c]["out"] for c in range(N_CORES)], axis=0)
    return out, res


def kernel(**inputs) -> np.ndarray:
    X = np.asarray(inputs["X"], dtype=np.float32)
    prep = _host_prep(
        X,
        inputs["dist_chunk"],
        np.asarray(inputs["non_missing_fix_X"]),
        np.asarray(inputs["mask_fit_X"]),
        np.asarray(inputs["dist_idx_map"]),
        np.asarray(inputs["mask"]),
        np.asarray(inputs["row_missing_idx"]),
        np.asarray(inputs["_fit_X"], dtype=np.float32),
    )
    if prep is None:
        return _numpy_reference(
            X,
            np.asarray(inputs["dist_chunk"], dtype=np.float32),
            np.asarray(inputs["non_missing_fix_X"]),
            np.asarray(inputs["mask_fit_X"]),
            np.asarray(inputs["dist_idx_map"]),
            np.asarray(inputs["mask"]),
            np.asarray(inputs["row_missing_idx"]),
            np.asarray(inputs["_fit_X"], dtype=np.float32),
        )
    dist_rows, receiver, pen_bf16, fitcol = prep
    out, _ = _run_on_device((dist_rows, X, receiver, pen_bf16, fitcol))
    return out.astype(np.float32)
